# revision 28
# baseline (speedup 1.0000x reference)
"""Multi-head attention (B=2, S=2048, D=1024, H=16, causal) on 8 TRN2 NeuronCores.

Sharding: core c handles batch c//4 and heads [4*(c%4), 4*(c%4)+4) —
data-parallel over batch x tensor-parallel over heads, Megatron-style:
QKV projection weights are column-split (each core computes only its own
heads' features), the output projection is row-split (each core emits a
full-width partial that the host sums).

Per-core device kernel (fp8 DoubleRow for the K-bound matmuls, bf16 for
the N-bound ones, fp32 accumulation):
  - q/k/v rows and Wq/Wk/Wv stream in as fp8e4 laid out in k-tile PAIRS
    ([128, 2, cols]) so Q/K/V projections run as DoubleRow matmuls
    (contract 256 per pass = 2x MAC rate, half the instructions) — EXCEPT
    seq [0:512], which streams in bf16 with bf16 weights: the few-key
    softmax rows of chunk 0 can't average away fp8 noise, and clean K/V
    for keys 0:512 also cleans every chunk's scores against those keys.
  - Q,K projected feature-major (QT/KT = W_local @ x^T, (256, 2048) bf16)
    so the scores matmul needs no on-device transposes; V projected in
    natural (seq, feat) layout with a fused ones-column so the PV matmul
    produces both attn@V and the softmax denominator Z.
  - scores^T per (head, q-chunk, key-chunk-pair) in bf16 (K=64 real
    features is N-bound; fp8 wouldn't speed it up).
  - softmax without max-subtraction; exp runs on ACT writing fp8e5 et
    directly (e5m2's range covers exp of the max ~8.5 score with no shift
    and never flushes realistic weights), one instruction per key-chunk
    PAIR (strided AP covers both halves); fully-masked leading columns of
    diagonal chunks are zeroed by widening the affine_select (negative
    base), not memset.
  - PV as fp8 DoubleRow: VA ([128, slot, kc, 128] fp8) pairs two key
    chunks per matmul; the ones/zero-pad column layout is unchanged so
    each head's output + Z row land on the right ON partition half.
  - normalization batched per CHUNK (4 heads at once): the 4 Z rows are
    gathered to partitions 0/32/64/96 of a shared tile (gpsimd; DVE for
    the latency-critical tail chunk), ONE 32x32 stream transpose spreads
    them across lanes, strided reciprocal, bf16 cast + transpose-back,
    then a K=128 one-hot bf16 matmul per head broadcasts 1/Z and DVE
    scales u into ON. One exp pair per chunk-3 head runs as a Schraudolph
    affine on DVE (uint8 bitcast e5m2) to shed ACT load.
  - O projection bf16 (full-output accuracy), partial output written
    feature-major (1024, 2048) fp16; host transposes/sums partials + bo.

Scheduling: input rows stream as separate pair/quarter tiles ordered
q.h0 / k.h0 / v.q0 / q.h1 / v.q1 / k.h1 (v h1 late); chunks run [0, 1,
3, 2] with the per-(chunk, head) filler schedule (remaining projections
fill chunks 0-1, O projections fill 3 and 2) to keep the PE dense for
the HAM clock gate; warm-up matmuls cover the initial DMA window; the
final O projection runs through the freed scores PSUM pool as m-block
pairs with ACT doing the output casts.
"""

import numpy as np
import ml_dtypes

import concourse.bacc as bacc
import concourse.mybir as mybir
import concourse.tile as tile
from concourse.bass_utils import run_bass_kernel_spmd

B, S, D, H = 2, 2048, 1024, 16
DK = D // H           # 64, head dim
DL = 256              # local (per-core) projected features = 4 heads
NHL = 4               # heads per core
NQ = 4                # q-chunks of 512
F32 = mybir.dt.float32
F16 = mybir.dt.float16
BF16 = mybir.dt.bfloat16
FP8 = mybir.dt.float8e4
E5 = mybir.dt.float8e5
U8 = mybir.dt.uint8
NPBF16 = ml_dtypes.bfloat16
NPFP8 = ml_dtypes.float8_e4m3
DRM = mybir.MatmulPerfMode.DoubleRow
# V/bv head-block permutation to slot order [h0, h2, h1, h3] (even-parity
# heads first): lets the PV matmul place each head's output directly in the
# partition half its ON slice needs.
VPERM = np.concatenate(
    [np.arange(64), np.arange(128, 192), np.arange(64, 128), np.arange(192, 256)]
)


def _emit(tc, io):
    nc = tc.nc
    qt, kt, vt = io["qt"], io["kt"], io["vt"]          # (1024, 2048) fp8
    qt16, kt16, vt16 = io["qt16"], io["kt16"], io["vt16"]  # (1024, 512) bf16
    wqt, wkt, wvt = io["wqt"], io["wkt"], io["wvt"]    # (128, 2048) fp8
    wqt16, wkt16, wvt16 = io["wqt16"], io["wkt16"], io["wvt16"]
    wot = io["wot"]                                    # (128, 2048) bf16
    bqc, bkc = io["bqc"], io["bkc"]                    # (128, 2) f32
    bvr = io["bvr"]                                    # (1, 256) bf16
    outp = io["outp"]                                  # (1024, 2048) f32
    EXP = mybir.ActivationFunctionType.Exp

    with (
        tc.tile_pool(name="const", bufs=1) as cw,
        tc.tile_pool(name="io", bufs=32) as iop,
        tc.tile_pool(name="big", bufs=1) as big,
        tc.tile_pool(name="work", bufs=3) as wk,
        tc.tile_pool(name="psA", bufs=2, space="PSUM") as psA,
        tc.tile_pool(name="psB", bufs=2, space="PSUM") as psB,
        tc.tile_pool(name="psC", bufs=2, space="PSUM") as psC,
    ):
        ones_sb = cw.tile([128, 128], BF16)
        nc.vector.memset(ones_sb[:], 1.0)
        # one-hot Z-broadcast lhsT: row 32h covers head h's 64-col block
        # (K=128 bf16; partition offsets stay 32-aligned for the BIR verifier)
        e4h = cw.tile([128, 256], BF16)
        nc.gpsimd.memset(e4h[:], 0.0)
        for hh in range(4):
            nc.gpsimd.memset(e4h[32 * hh : 32 * hh + 1, hh * 64 : (hh + 1) * 64], 1.0)
        # Z-row gather tiles (head h's Z on partition 32h) for the per-chunk
        # batched normalize; two statics alternate between in-flight chunks
        # (order 0,1,3,2 -> A,B,A,B) so unwritten rows stay deterministic
        zrowA = cw.tile([128, 512], F32)
        nc.gpsimd.memset(zrowA[:], 0.0)
        zrowB = cw.tile([128, 512], F32)
        nc.gpsimd.memset(zrowB[:], 0.0)
        bq_sb = cw.tile([128, 2], F32)
        nc.sync.dma_start(bq_sb[:], bqc[:, :])
        bk_sb = cw.tile([128, 2], F32)
        nc.sync.dma_start(bk_sb[:], bkc[:, :])
        bv_sb = cw.tile([1, 512], BF16)
        nc.sync.dma_start(bv_sb[:], bvr[:, :])

        wq_sb = cw.tile([128, 8, 256], FP8)

        # free PE warm-up: dependency-less matmuls run while the first
        # weight/row DMAs are in flight, so the HAM clock gate is already at
        # 8/8 when the real work begins
        warm = cw.tile([128, 512], BF16, name="warm")
        nc.vector.memset(warm[:], 0.0)
        for _ in range(4):
            wps = psC.tile([128, 512], F32, tag="pv", name="wps")
            nc.tensor.matmul(wps[:], ones_sb[:, :], warm[:], start=True, stop=True)

        QT = big.tile([128, 2, S], BF16)   # [feat%128, feat//128, seq]
        # K^T kept as two half-zeroed copies so the scores matmul contracts
        # over the full 128 partitions (zeros kill the other head's Q rows);
        # K=64 matmuls read as "half-idle" to the PE activity monitor and the
        # clock gate kept re-throttling the whole attention phase.
        KTe = big.tile([128, 2, S], BF16)
        KTo = big.tile([128, 2, S], BF16)
        # [key%128, slot, key//128, 128 cols] fp8: slots 0:2 hold the
        # even-parity heads (h=0,2) with feats in cols 0:64 + ones col at 64
        # (Z lands on pvp row 64); slots 2:4 hold odd-parity heads (h=1,3)
        # with feats in cols 64:128 + ones col at 0 (Z on pvp row 0) so the
        # PV output lands directly in the ON partition half it belongs to.
        # Padded to 128 cols so PV matmuls drive the full array.
        VA = big.tile([128, NHL, 16, 128], FP8)
        ON = big.tile([128, 2, S], BF16)   # normalized attn out, feature-major
        nc.gpsimd.memset(VA[:, 0:2, :, 64:128], 0.0)
        nc.gpsimd.memset(VA[:, 0:2, :, 64:65], 1.0)
        nc.gpsimd.memset(VA[:, 2:4, :, 0:64], 0.0)
        nc.gpsimd.memset(VA[:, 2:4, :, 0:1], 1.0)
        # bf16 copy of the first 4 key chunks for chunk 0's bf16 PV path
        VAB = big.tile([128, NHL, 4, 128], BF16)
        nc.gpsimd.memset(VAB[:, 0:2, :, 64:128], 0.0)
        nc.gpsimd.memset(VAB[:, 0:2, :, 64:65], 1.0)
        nc.gpsimd.memset(VAB[:, 2:4, :, 0:64], 0.0)
        nc.gpsimd.memset(VAB[:, 2:4, :, 0:1], 1.0)
        # K^T zero-halves AFTER the VA/VAB sets in the gpsimd queue: the VA
        # ones columns gate the first PV, the KT zeros only the first scores
        nc.gpsimd.memset(KTe[64:128, :, :], 0.0)
        nc.gpsimd.memset(KTo[0:64, :, :], 0.0)

        # ---- input row DMAs. Seq [0:512] of q/k/v streams in BF16 (with
        # bf16 weight copies): chunk 0's few-key softmax rows can't average
        # away fp8 noise, and clean K/V for keys 0:512 also cleans every
        # later chunk's scores against those keys. Seq [512:2048] streams as
        # fp8 k-tile PAIR tiles [128, 2, cols] (DoubleRow layout). Ordered by
        # first consumer; the v h1 halves are emitted late ----
        wq16_sb = cw.tile([128, 8, 256], BF16)
        wk16_sb = cw.tile([128, 8, 256], BF16)
        wv16_sb = cw.tile([128, 8, 256], BF16)
        wk_sb = cw.tile([128, 8, 256], FP8)
        wv_sb = cw.tile([128, 8, 256], FP8)
        wo_sb = cw.tile([128, 2, 1024], BF16)
        HS = 1024
        q16rows = [None] * 8   # [ktile] -> [128, 512] bf16 (seq 0:512)
        k16rows = [None] * 8
        v16rows = [None] * 8
        qrows = [[None, None] for _ in range(4)]   # [pair][0: seq 512:1024, 1: 1024:2048]
        krows = [[None, None] for _ in range(4)]
        vrows = [[None, None] for _ in range(4)]   # [pair][q1 512:1024, h1 1024:2048]

        def row16_dma(rows, src16, k, nm):
            r = iop.tile([128, 512], BF16, tag="x16", name=f"{nm}{k}", bufs=24)
            nc.sync.dma_start(r[:], src16[k * 128 : (k + 1) * 128, :])
            rows[k] = r

        def row_dma(rows, src, p, hf, nm):
            # hf 0: seq [512:1024] (w 512); hf 1: seq [1024:2048] (w 1024)
            w = 512 if hf == 0 else HS
            tg = "xq" if hf == 0 else "xrh"
            r = iop.tile([128, 2, w], FP8, tag=tg, name=f"{nm}{p}h{hf}", bufs=12)
            base = 512 if hf == 0 else 1024
            nc.sync.dma_start(
                r[:],
                src[p * 256 : (p + 1) * 256, base : base + w].rearrange(
                    "(i p) n -> p i n", i=2
                ),
            )
            rows[p][hf] = r

        nc.sync.dma_start(wq16_sb[:], wqt16[:, :].rearrange("p (k m) -> p k m", m=256))
        for k in range(8):
            row16_dma(q16rows, qt16, k, "q16r")
        nc.sync.dma_start(wk16_sb[:], wkt16[:, :].rearrange("p (k m) -> p k m", m=256))
        for k in range(8):
            row16_dma(k16rows, kt16, k, "k16r")
        nc.sync.dma_start(wv16_sb[:], wvt16[:, :].rearrange("p (k m) -> p k m", m=256))
        for k in range(8):
            row16_dma(v16rows, vt16, k, "v16r")
        nc.sync.dma_start(wq_sb[:], wqt[:, :].rearrange("p (k m) -> p k m", m=256))
        for p in range(4):
            row_dma(qrows, qt, p, 0, "qr")
        nc.sync.dma_start(wk_sb[:], wkt[:, :].rearrange("p (k m) -> p k m", m=256))
        for p in range(4):
            row_dma(krows, kt, p, 0, "kr")
        nc.sync.dma_start(wv_sb[:], wvt[:, :].rearrange("p (k m) -> p k m", m=256))
        for p in range(4):
            row_dma(vrows, vt, p, 0, "vr")
        for p in range(4):
            row_dma(qrows, qt, p, 1, "qr")
        for p in range(4):
            row_dma(krows, kt, p, 1, "kr")

        def xcol(rows, p, c0, w=512):  # [128, 2, w] fp8 slice (c0 >= 512)
            if c0 < 1024:
                return rows[p][0][:, :, c0 - 512 : c0 - 512 + w]
            return rows[p][1][:, :, c0 - 1024 : c0 - 1024 + w]

        def vcol(p, c0, w=128):  # fp8 v slice (c0 >= 512)
            if c0 < 1024:
                return vrows[p][0][:, :, c0 - 512 : c0 - 512 + w]
            return vrows[p][1][:, :, c0 - 1024 : c0 - 1024 + w]

        # ---- Q/K projections, feature-major: chunk 0 in bf16, chunks 1-3 as
        # fp8 DoubleRow (contract 2 k-tiles per matmul) ----
        def emit_qproj(n):
            pm = [
                psA.tile([128, 512], F32, tag="proj", name=f"pm{m}")
                for m in range(2)
            ]
            if n == 0:
                for k in range(8):
                    for m in range(2):
                        nc.tensor.matmul(
                            pm[m][:],
                            wq16_sb[:, k, m * 128 : (m + 1) * 128],
                            q16rows[k][:],
                            start=(k == 0),
                            stop=(k == 7),
                        )
            else:
                for p in range(4):
                    for m in range(2):
                        nc.tensor.matmul(
                            pm[m][:],
                            wq_sb[:, 2 * p : 2 * p + 2, m * 128 : (m + 1) * 128],
                            xcol(qrows, p, n * 512),
                            start=(p == 0),
                            stop=(p == 3),
                            perf_mode=DRM,
                        )
            for m in range(2):
                nc.vector.tensor_scalar_add(
                    QT[:, m, n * 512 : (n + 1) * 512], pm[m][:], bq_sb[:, m : m + 1]
                )

        # K projection split per q-chunk: attention chunk j only needs K
        # columns up to (j+1)*512, so later chunks are emitted between the
        # attention chunks below (PE-dense filler for the exp-paced phase).
        # Bias adds stay on DVE: GpSimd cannot read PSUM.
        def emit_kproj(n):
            pm = [
                psA.tile([128, 512], F32, tag="proj", name=f"km{m}")
                for m in range(2)
            ]
            if n == 0:
                for k in range(8):
                    for m in range(2):
                        nc.tensor.matmul(
                            pm[m][:],
                            wk16_sb[:, k, m * 128 : (m + 1) * 128],
                            k16rows[k][:],
                            start=(k == 0),
                            stop=(k == 7),
                        )
            else:
                for p in range(4):
                    for m in range(2):
                        nc.tensor.matmul(
                            pm[m][:],
                            wk_sb[:, 2 * p : 2 * p + 2, m * 128 : (m + 1) * 128],
                            xcol(krows, p, n * 512),
                            start=(p == 0),
                            stop=(p == 3),
                            perf_mode=DRM,
                        )
            for m in range(2):
                sl = slice(n * 512, (n + 1) * 512)
                nc.vector.tensor_scalar_add(
                    KTe[0:64, m, sl], pm[m][0:64, :], bk_sb[0:64, m : m + 1]
                )
                nc.vector.tensor_scalar_add(
                    KTo[64:128, m, sl], pm[m][64:128, :], bk_sb[64:128, m : m + 1]
                )

        emit_qproj(0)
        emit_kproj(0)
        for _ in range(8):  # filler: the v16 rows may still be in flight
            wps = psC.tile([128, 512], F32, tag="pv", name="wps")
            nc.tensor.matmul(wps[:], ones_sb[:, :], warm[:], start=True, stop=True)
        nc.sync.dma_start(wo_sb[:], wot[:, :].rearrange("p (c m) -> p c m", m=1024))

        # ---- V projection, natural layout, DoubleRow (x rows stationary),
        # bias via K=1 bf16 ones matmul; emitted in sp-pairs interleaved with
        # the attention chunks below as PE-dense filler ----
        def emit_vproj(sps):
            for sp in sps:
                pvps = psA.tile([128, 512], F32, tag="proj", name="pvps")
                if sp < 2:  # seq 0:512 in bf16 (x rows stationary)
                    for half in range(2):
                        s = sp * 256 + half * 128
                        for k in range(8):
                            nc.tensor.matmul(
                                pvps[:, half * 256 : (half + 1) * 256],
                                v16rows[k][:, s : s + 128],
                                wv16_sb[:, k, :],
                                start=(k == 0 and half == 0),
                                stop=False,
                                skip_group_check=(half == 1),
                            )
                else:
                    for half in range(2):
                        s = sp * 256 + half * 128
                        for p in range(4):
                            nc.tensor.matmul(
                                pvps[:, half * 256 : (half + 1) * 256],
                                vcol(p, s),
                                wv_sb[:, 2 * p : 2 * p + 2, :],
                                start=(p == 0 and half == 0),
                                stop=False,
                                perf_mode=DRM,
                                skip_group_check=(half == 1),
                            )
                nc.tensor.matmul(
                    pvps[:, 0:512],
                    ones_sb[0:1, 0:128],
                    bv_sb[:],
                    start=False,
                    stop=True,
                    skip_group_check=True,
                )
                # wvt/bvr are host-permuted to slot order [h0, h2, h1, h3]:
                # slots 0:2 (even-parity heads) land in VA cols 0:64, slots
                # 2:4 (odd-parity) in cols 64:128 — matching the pvp row
                # ranges their ON partition halves need.
                for half in range(2):
                    s = sp * 2 + half
                    nc.vector.tensor_copy(
                        VA[:, 0:2, s, 0:64],
                        pvps[:, half * 256 : half * 256 + 128].rearrange(
                            "p (h d) -> p h d", d=64
                        ),
                    )
                    nc.vector.tensor_copy(
                        VA[:, 2:4, s, 64:128],
                        pvps[:, half * 256 + 128 : (half + 1) * 256].rearrange(
                            "p (h d) -> p h d", d=64
                        ),
                    )
                    if s < 4:  # chunk 0's bf16 PV needs these in bf16 too
                        nc.vector.tensor_copy(
                            VAB[:, 0:2, s, 0:64],
                            pvps[:, half * 256 : half * 256 + 128].rearrange(
                                "p (h d) -> p h d", d=64
                            ),
                        )
                        nc.vector.tensor_copy(
                            VAB[:, 2:4, s, 64:128],
                            pvps[:, half * 256 + 128 : (half + 1) * 256].rearrange(
                                "p (h d) -> p h d", d=64
                            ),
                        )

        # ---- attention + output projection, q-chunk-major for overlap ----
        # (vproj(0,1) is emitted inside head (0,0), between its scores/exp
        # and its PV matmuls, so the scores don't queue behind the v16 DMA)

        def emit_oproj(j, ms=range(8)):
            for m in ms:
                po = psA.tile([128, 512], F32, tag="proj", name="po")
                for c in range(2):
                    nc.tensor.matmul(
                        po[:],
                        wo_sb[:, c, m * 128 : (m + 1) * 128],
                        ON[:, c, j * 512 : (j + 1) * 512],
                        start=(c == 0),
                        stop=(c == 1),
                    )
                ot = wk.tile([128, 512], F16, tag="ot", name="ot")
                nc.vector.tensor_copy(ot[:], po[:])
                nc.sync.dma_start(
                    outp[m * 128 : (m + 1) * 128, j * 512 : (j + 1) * 512], ot[:]
                )

        # normalize chunk j (batched over its 4 heads): gpsimd gathers the 4
        # Z rows into zrow, ONE 32x32 stream-transpose spreads all of them
        # across 32 lanes, reciprocal runs on a strided view (4 cols per
        # 32-block), bf16 cast + transpose-back, then a K=32 one-hot bf16
        # matmul per head broadcasts 1/Z across the head's 64 partitions and
        # DVE scales u into ON. Odd-parity heads produced their PV output
        # directly on partitions 64:128 (VA slot layout), so both parities
        # write ON in place.
        def emit_norm_chunk(j, us):
            sl = slice(j * 512, (j + 1) * 512)
            zrow = zrowA if j in (0, 3) else zrowB
            zeng = nc.vector if j == 2 else nc.gpsimd
            for h in range(4):
                zb = 64 if h % 2 == 0 else 0
                zeng.tensor_copy(zrow[32 * h : 32 * h + 1, :], us[h][zb : zb + 1, :])
            zt = wk.tile([128, 512], F32, tag="zt", name="zt", bufs=2)
            nc.vector.transpose(zt[:, :], zrow[:, :])
            nc.vector.reciprocal(zt[:, 0:512:32], zt[:, 0:512:32])
            ztb = wk.tile([128, 512], BF16, tag="ztb", name="ztb", bufs=2)
            nc.vector.tensor_copy(ztb[:, :], zt[:, :])
            zbb = wk.tile([128, 512], BF16, tag="zbb", name="zbb", bufs=2)
            nc.vector.transpose(zbb[:, :], ztb[:, :])
            for ch in range(2):
                he, ho = 2 * ch, 2 * ch + 1
                rbt = psA.tile([128, 512], F32, tag="proj", name="rbt")
                nc.tensor.matmul(
                    rbt[0:64, :], e4h[:, he * 64 : (he + 1) * 64], zbb[:, :],
                    start=True, stop=True,
                )
                nc.tensor.matmul(
                    rbt[64:128, :], e4h[:, ho * 64 : (ho + 1) * 64], zbb[:, :],
                    start=True, stop=True, skip_group_check=True,
                )
                nc.vector.tensor_mul(ON[0:64, ch, sl], us[he][0:64, :], rbt[0:64, :])
                nc.vector.tensor_mul(
                    ON[64:128, ch, sl], us[ho][64:128, :], rbt[64:128, :]
                )

        # half-chunk normalize for the tail chunk: heads (0,1) [pb=0] or
        # (2,3) [pb=64] only — halves the final DVE chain so the tail
        # O-projection starts sooner. Partition math mirrors the full chunk
        # version; all offsets stay 32-aligned.
        def emit_norm_half(j, heads, us):
            sl = slice(j * 512, (j + 1) * 512)
            zrow = zrowA if j in (0, 3) else zrowB
            pb = 0 if heads[0] == 0 else 64
            for h in heads:
                zb = 64 if h % 2 == 0 else 0
                nc.vector.tensor_copy(
                    zrow[32 * h : 32 * h + 1, :], us[h][zb : zb + 1, :]
                )
            zt = wk.tile([128, 512], F32, tag="zt", name="zt", bufs=2)
            nc.vector.transpose(zt[pb : pb + 64, :], zrow[pb : pb + 64, :])
            nc.vector.reciprocal(
                zt[pb : pb + 64, 0:512:32], zt[pb : pb + 64, 0:512:32]
            )
            ztb = wk.tile([128, 512], BF16, tag="ztb", name="ztb", bufs=2)
            nc.vector.tensor_copy(ztb[pb : pb + 64, :], zt[pb : pb + 64, :])
            zbb = wk.tile([128, 512], BF16, tag="zbb", name="zbb", bufs=2)
            nc.vector.transpose(zbb[pb : pb + 64, :], ztb[pb : pb + 64, :])
            he, ho = heads
            ch = he // 2
            rbt = psA.tile([128, 512], F32, tag="proj", name="rbt")
            nc.tensor.matmul(
                rbt[0:64, :], e4h[pb : pb + 64, he * 64 : (he + 1) * 64],
                zbb[pb : pb + 64, :], start=True, stop=True,
            )
            nc.tensor.matmul(
                rbt[64:128, :], e4h[pb : pb + 64, ho * 64 : (ho + 1) * 64],
                zbb[pb : pb + 64, :], start=True, stop=True,
                skip_group_check=True,
            )
            nc.vector.tensor_mul(ON[0:64, ch, sl], us[he][0:64, :], rbt[0:64, :])
            nc.vector.tensor_mul(
                ON[64:128, ch, sl], us[ho][64:128, :], rbt[64:128, :]
            )

        # PE filler schedule: chunks run in order [0, 1, 3, 2] so the big
        # ACT-paced late chunks have O-projections available as filler (chunk
        # 2 last gets oproj(3)); chunk 0's fillers finish the Q projection
        # (attention j only reads QT chunk j) and the K/V pipeline. Each
        # (chunk, head) entry is emitted right after that head's PV drain.
        def fillers(j, h):
            if (j, h) == (0, 0):
                emit_qproj(1)
            elif (j, h) == (0, 1):
                emit_kproj(1)
            elif (j, h) == (0, 2):
                emit_qproj(2)
            elif (j, h) == (0, 3):
                for p in range(4):  # late v halves recycle released q slots
                    r = iop.tile([128, 2, HS], FP8, tag="xrh", name=f"vr{p}h1", bufs=12)
                    nc.sync.dma_start(
                        r[:],
                        vt[p * 256 : (p + 1) * 256, 1024:2048].rearrange(
                            "(i p) n -> p i n", i=2
                        ),
                    )
                    vrows[p][1] = r
                emit_vproj((2, 3))
            elif (j, h) == (1, 0):
                emit_kproj(2)
            elif (j, h) == (1, 1):
                emit_qproj(3)
            elif (j, h) == (1, 2):
                emit_kproj(3)
            elif (j, h) == (1, 3):
                emit_vproj((4, 5, 6, 7))
            elif (j, h) == (3, 0):
                emit_oproj(0, range(4))
            elif (j, h) == (3, 1):
                emit_oproj(0, range(4, 8))
            elif (j, h) == (3, 2):
                emit_oproj(1, range(4))
            elif (j, h) == (3, 3):
                emit_oproj(1, range(4, 8))
                for _ in range(6):  # cover norm(3)'s DVE-chain wait
                    wps = psC.tile([128, 512], F32, tag="pv", name="wps")
                    nc.tensor.matmul(
                        wps[:], ones_sb[:, :], warm[:], start=True, stop=True
                    )
            elif (j, h) == (2, 0):
                emit_oproj(3, range(4))
            elif (j, h) == (2, 1):
                emit_oproj(3, range(4, 8))
            elif (j, h) in ((2, 2), (2, 3)):
                # dependency-free PE keep-alive: bridges the chunk-2 norm
                # window so the HAM clock gate doesn't throttle the tail
                # O-projection to half speed
                for _ in range(6 if h == 2 else 26):
                    wps = psC.tile([128, 512], F32, tag="pv", name="wps")
                    nc.tensor.matmul(
                        wps[:], ones_sb[:, :], warm[:], start=True, stop=True
                    )

        # chunks 0 and 1 run head-sequentially (their fillers are the
        # remaining projections); chunks 3 and 2 INTERLEAVE heads so their
        # pooled PE deficit draws on the pooled O-projection filler
        SEQ = [(0, h) for h in range(NHL)] + [(1, h) for h in range(NHL)]
        SEQ += [(3, h) for h in range(NHL)] + [(2, h) for h in range(NHL)]
        # norm of chunk c is emitted after the u-copy at position (j, h) —
        # late enough not to block the next chunk's VA/bias DVE work, early
        # enough that the O-projection fillers find ON ready
        NORM_AT = {(1, 0): 0, (3, 0): 1, (3, 3): 3}
        # (3,3) norm runs before chunk 2's attention begins, so the oproj(3)
        # fillers at (2,0)/(2,1) find ON(3) ready
        uchunk = {}
        for j, h in SEQ:
            nkc = 4 * (j + 1)
            ch = h // 2
            slot = (h % 2) * 2 + ch
            pvp = psC.tile([128, 512], F32, tag="pv", name="pvp")
            first = (j, h) == (0, 0)
            ets = {}
            for kp in range(nkc // 2):
                scp = psB.tile([128, 1024], F32, tag="sc", name="scp")
                ktz = KTe if h % 2 == 0 else KTo
                # q0 per half (diagonal chunk t = kc-4j only reaches q >=
                # 128t); the PAIR is processed over [q0p:512] where q0p is
                # the first half's q0 — the widened affine_select (negative
                # base) zeroes the second half's fully-masked leading cols.
                q0 = [128 * max(0, 2 * kp + half - 4 * j) for half in range(2)]
                q0p = q0[0]
                for half in range(2):
                    kc = 2 * kp + half
                    nc.tensor.matmul(
                        scp[:, half * 512 + q0[half] : (half + 1) * 512],
                        ktz[:, ch, kc * 128 : (kc + 1) * 128],
                        QT[:, ch, j * 512 + q0[half] : (j + 1) * 512],
                        start=True,
                        stop=True,
                    )
                # chunk 0's q rows see few keys, so softmax averaging can't
                # wash out fp8 quantization noise — use bf16 et + per-kc bf16
                # PV (VAB) for j=0; fp8e5 et + DoubleRow PV elsewhere
                edt = BF16 if j == 0 else E5
                et = wk.tile([128, 1024], edt, tag="exp" + ("16" if j == 0 else ""),
                             name="et", bufs=4 if j == 0 else 8)
                if q0p == 0 and q0[1] == 0:
                    if j == 3 and kp == 5:
                        # Schraudolph exp on DVE (affine -> uint8, bitcast
                        # e5m2; negatives saturate to +0) — sheds one ACT
                        # instruction per chunk-3 head onto the idler DVE
                        nc.vector.tensor_scalar(
                            et[:].bitcast(U8), scp[:], 0.7213475, 59.8,
                            mybir.AluOpType.mult, mybir.AluOpType.add,
                        )
                    else:
                        nc.scalar.activation(
                            et[:], scp[:], EXP, scale=float(DK) ** -0.5
                        )
                else:
                    # one ACT instruction covers both halves over [q0p:512]
                    # (strided 2D-free AP); the [q0p:q0[1]] region of the
                    # second half is exp(PSUM garbage) — finite, and zeroed
                    # by the widened select below
                    etv = et[:, :].rearrange("p (c n) -> p c n", c=2)
                    scv = scp[:, :].rearrange("p (c n) -> p c n", c=2)
                    nc.scalar.activation(
                        etv[:, :, q0p:512],
                        scv[:, :, q0p:512],
                        EXP,
                        scale=float(DK) ** -0.5,
                    )
                for half in range(2):
                    kc = 2 * kp + half
                    if kc >= 4 * j:
                        t = kc - 4 * j
                        nc.gpsimd.affine_select(
                            out=et[:, half * 512 + q0p : (half + 1) * 512],
                            in_=et[:, half * 512 + q0p : (half + 1) * 512],
                            compare_op=mybir.AluOpType.is_ge,
                            fill=0.0,
                            base=q0p - 128 * t,
                            pattern=[[1, 512 - q0p]],
                            channel_multiplier=-1,
                        )
                if first:
                    ets[kp] = (et, list(q0))
                    continue
                if j == 0:
                    for half in range(2):
                        kc = 2 * kp + half
                        nc.tensor.matmul(
                            pvp[:, q0[half] : 512],
                            VAB[:, slot, kc, :],
                            et[:, half * 512 + q0[half] : (half + 1) * 512],
                            start=(kc == 0),
                            stop=(kc == nkc - 1),
                            skip_group_check=(q0[half] > 0),
                        )
                else:
                    # PV DoubleRow: one matmul contracts both key chunks of
                    # the pair (VA slot already holds [keys, kc, 128cols] fp8)
                    etv = et[:, :].rearrange("p (c n) -> p c n", c=2)
                    nc.tensor.matmul(
                        pvp[:, q0p:512],
                        VA[:, slot, 2 * kp : 2 * kp + 2, :],
                        etv[:, :, q0p:512],
                        start=(kp == 0),
                        stop=(kp == nkc // 2 - 1),
                        perf_mode=DRM,
                        skip_group_check=(q0p > 0),
                    )
            if first:
                emit_vproj((0, 1))
                for kp in range(nkc // 2):
                    et, q0 = ets[kp]
                    for half in range(2):
                        kc = 2 * kp + half
                        nc.tensor.matmul(
                            pvp[:, q0[half] : 512],
                            VAB[:, slot, kc, :],
                            et[:, half * 512 + q0[half] : (half + 1) * 512],
                            start=(kc == 0),
                            stop=(kc == nkc - 1),
                            skip_group_check=(q0[half] > 0),
                        )
            # copy the PV accumulator to SBUF immediately — freeing the
            # pv PSUM slot fast keeps the PE from stalling. The normalize is
            # deferred by ONE head so the next head's PSUM-freeing copy is
            # never queued behind it on DVE.
            u = wk.tile([128, 512], F32, tag="u", name="u", bufs=9)
            nc.vector.tensor_copy(u[:], pvp[:])
            uchunk.setdefault(j, {})[h] = u
            # fillers BEFORE the norm: the norm's broadcast matmuls wait on
            # the DVE transpose/recip chain, and the in-order PE queue would
            # stall the filler matmuls behind them
            fillers(j, h)
            if (j, h) in NORM_AT:
                emit_norm_chunk(NORM_AT[(j, h)], uchunk.pop(NORM_AT[(j, h)]))
            elif (j, h) == (2, 1):
                emit_norm_half(2, (0, 1), uchunk[2])
            elif (j, h) == (2, 3):
                emit_norm_half(2, (2, 3), uchunk.pop(2))
        # keep-alive while chunk 2's ON multiplies drain on DVE (the first
        # tail matmul below waits on them; an idle PE would re-throttle)
        for _ in range(6):
            wps = psC.tile([128, 512], F32, tag="pv", name="wps")
            nc.tensor.matmul(wps[:], ones_sb[:, :], warm[:], start=True, stop=True)
        # last chunk's (j=2) O-projection: psB (the scores pool) is free now,
        # so build m-block PAIRS in [128,1024] tiles — half the copies/DMAs
        # and 2KB/partition output descriptors — to compress the serial tail.
        j = 2
        for mp in range(4):
            po = psB.tile([128, 1024], F32, tag="sc", name="po2")
            for i in range(2):
                m = 2 * mp + i
                for c in range(2):
                    nc.tensor.matmul(
                        po[:, i * 512 : (i + 1) * 512],
                        wo_sb[:, c, m * 128 : (m + 1) * 128],
                        ON[:, c, j * 512 : (j + 1) * 512],
                        start=(c == 0),
                        stop=(c == 1),
                    )
            ot = wk.tile([128, 1024], F16, tag="ot2", name="ot2", bufs=2)
            nc.scalar.copy(ot[:], po[:])  # ACT is idle at the tail; DVE isn't
            nc.sync.dma_start(
                outp[
                    2 * mp * 128 : (2 * mp + 2) * 128, j * 512 : (j + 1) * 512
                ].rearrange("(b p) c -> p b c", p=128),
                ot[:].rearrange("p (b c) -> p b c", c=512),
            )


def build_nc():
    nc = bacc.Bacc("TRN2", target_bir_lowering=False, debug=False, num_devices=8)
    io = {}
    for name, shape, dt in (
        ("qt", (D, S), FP8),
        ("kt", (D, S), FP8),
        ("vt", (D, S), FP8),
        ("qt16", (D, 512), BF16),
        ("kt16", (D, 512), BF16),
        ("vt16", (D, 512), BF16),
        ("wqt16", (128, 2048), BF16),
        ("wkt16", (128, 2048), BF16),
        ("wvt16", (128, 2048), BF16),
        ("wqt", (128, 2048), FP8),
        ("wkt", (128, 2048), FP8),
        ("wvt", (128, 2048), FP8),
        ("wot", (128, 2048), BF16),
        ("bqc", (128, 2), F32),
        ("bkc", (128, 2), F32),
        ("bvr", (1, 512), BF16),
    ):
        io[name] = nc.dram_tensor(name, shape, dt, kind="ExternalInput")
    io["outp"] = nc.dram_tensor("outp", (D, S), F16, kind="ExternalOutput")
    with tile.TileContext(nc) as tc:
        _emit(tc, io)
    nc.compile()
    return nc


_NC = None


def _get_nc():
    global _NC
    if _NC is None:
        _NC = build_nc()
    return _NC


def make_in_maps(q, k, v, Wq, bq, Wk, bk, Wv, bv, Wo):
    def c8(x):  # contiguous fp8e4
        return np.ascontiguousarray(x).astype(NPFP8)

    def cb(x):  # contiguous bf16
        return np.ascontiguousarray(x).astype(NPBF16)

    cf = np.ascontiguousarray
    in_maps = []
    for core in range(8):
        b, g = divmod(core, 4)
        sl = slice(DL * g, DL * (g + 1))
        in_maps.append(
            {
                "qt": c8(q[b].T),
                "kt": c8(k[b].T),
                "vt": c8(v[b].T),
                "qt16": cb(q[b].T[:, 0:512]),
                "kt16": cb(k[b].T[:, 0:512]),
                "vt16": cb(v[b].T[:, 0:512]),
                "wqt16": cb(Wq[sl, :].T.reshape(8, 128, DL).transpose(1, 0, 2).reshape(128, 2048)),
                "wkt16": cb(Wk[sl, :].T.reshape(8, 128, DL).transpose(1, 0, 2).reshape(128, 2048)),
                "wvt16": cb(Wv[sl, :][VPERM, :].T.reshape(8, 128, DL).transpose(1, 0, 2).reshape(128, 2048)),
                "wqt": c8(Wq[sl, :].T.reshape(8, 128, DL).transpose(1, 0, 2).reshape(128, 2048)),
                "wkt": c8(Wk[sl, :].T.reshape(8, 128, DL).transpose(1, 0, 2).reshape(128, 2048)),
                "wvt": c8(Wv[sl, :][VPERM, :].T.reshape(8, 128, DL).transpose(1, 0, 2).reshape(128, 2048)),
                "wot": cb(Wo[:, sl].T.reshape(2, 128, D).transpose(1, 0, 2).reshape(128, 2048)),
                "bqc": cf(bq[sl].reshape(2, 128).T),
                "bkc": cf(bk[sl].reshape(2, 128).T),
                "bvr": cb(np.tile(bv[sl][VPERM], 2).reshape(1, 512)),
            }
        )
    return in_maps


def gather_output(results, bo):
    out = np.empty((B, S, D), np.float32)
    for b in range(B):
        acc = results[4 * b]["outp"].astype(np.float32)
        for g in range(1, 4):
            acc = acc + results[4 * b + g]["outp"]
        out[b] = acc.T + bo
    return out


def _np_fallback(q, k, v, mask, Wq, bq, Wk, bk, Wv, bv, Wo, bo):
    # generic-mask reference path; only used if the mask is not causal
    out = np.empty((B, S, D), np.float32)
    m = np.broadcast_to(mask, (B, 1, S, S))
    for b in range(B):
        Q = (q[b] @ Wq.T + bq).reshape(S, H, DK).transpose(1, 0, 2)
        K = (k[b] @ Wk.T + bk).reshape(S, H, DK).transpose(1, 0, 2)
        V = (v[b] @ Wv.T + bv).reshape(S, H, DK).transpose(1, 0, 2)
        o = np.empty((H, S, DK), np.float32)
        for hh in range(H):
            s = (Q[hh] @ K[hh].T) * (DK**-0.5)
            s = np.where(m[b, 0] == 0, -np.inf, s)
            s = s - s.max(axis=-1, keepdims=True)
            e = np.exp(s)
            o[hh] = (e / e.sum(axis=-1, keepdims=True)) @ V[hh]
        out[b] = o.transpose(1, 0, 2).reshape(S, D) @ Wo.T + bo
    return out


def kernel(q, k, v, mask, Wq, bq, Wk, bk, Wv, bv, Wo, bo):
    f32 = np.float32
    q, k, v = (np.asarray(x, f32) for x in (q, k, v))
    Wq, bq, Wk, bk = (np.asarray(x, f32) for x in (Wq, bq, Wk, bk))
    Wv, bv, Wo, bo = (np.asarray(x, f32) for x in (Wv, bv, Wo, bo))
    mask = np.asarray(mask)

    if not np.array_equal(
        np.broadcast_to(mask, (1, 1, S, S))[0, 0] != 0,
        np.tril(np.ones((S, S), bool)),
    ):
        return _np_fallback(q, k, v, mask, Wq, bq, Wk, bk, Wv, bv, Wo, bo)

    nc = _get_nc()
    in_maps = make_in_maps(q, k, v, Wq, bq, Wk, bk, Wv, bv, Wo)
    res = run_bass_kernel_spmd(nc, in_maps, list(range(8)))
    return gather_output(res.results, bo)


# revision 29
# speedup vs baseline: 1.0106x; 1.0106x over previous
"""Multi-head attention (B=2, S=2048, D=1024, H=16, causal) on 8 TRN2 NeuronCores.

Sharding: core c handles batch c//4 and heads [4*(c%4), 4*(c%4)+4) —
data-parallel over batch x tensor-parallel over heads, Megatron-style:
QKV projection weights are column-split (each core computes only its own
heads' features), the output projection is row-split (each core emits a
full-width partial that the host sums).

Per-core device kernel (fp8 DoubleRow for the K-bound matmuls, bf16 for
the N-bound ones, fp32 accumulation):
  - q/k/v rows and Wq/Wk/Wv stream in as fp8e4 laid out in k-tile PAIRS
    ([128, 2, cols]) so Q/K/V projections run as DoubleRow matmuls
    (contract 256 per pass = 2x MAC rate, half the instructions) — EXCEPT
    seq [0:512], which streams in bf16 with bf16 weights: the few-key
    softmax rows of chunk 0 can't average away fp8 noise, and clean K/V
    for keys 0:512 also cleans every chunk's scores against those keys.
  - Q,K projected feature-major (QT/KT = W_local @ x^T, (256, 2048) bf16)
    so the scores matmul needs no on-device transposes; V projected in
    natural (seq, feat) layout with a fused ones-column so the PV matmul
    produces both attn@V and the softmax denominator Z.
  - scores^T per (head, q-chunk, key-chunk-pair) in bf16 (K=64 real
    features is N-bound; fp8 wouldn't speed it up).
  - softmax without max-subtraction; exp runs on ACT writing fp8e5 et
    directly (e5m2's range covers exp of the max ~8.5 score with no shift
    and never flushes realistic weights), one instruction per key-chunk
    PAIR (strided AP covers both halves); fully-masked leading columns of
    diagonal chunks are zeroed by widening the affine_select (negative
    base), not memset.
  - PV as fp8 DoubleRow: VA ([128, slot, kc, 128] fp8) pairs two key
    chunks per matmul; the ones/zero-pad column layout is unchanged so
    each head's output + Z row land on the right ON partition half.
  - normalization batched per CHUNK (4 heads at once): the 4 Z rows are
    gathered to partitions 0/32/64/96 of a shared tile (gpsimd; DVE for
    the latency-critical tail chunk), ONE 32x32 stream transpose spreads
    them across lanes, strided reciprocal, bf16 cast + transpose-back,
    then a K=128 one-hot bf16 matmul per head broadcasts 1/Z and DVE
    scales u into ON. One exp pair per chunk-3 head runs as a Schraudolph
    affine on DVE (uint8 bitcast e5m2) to shed ACT load.
  - O projection bf16 (full-output accuracy), partial output written
    feature-major (1024, 2048) fp16; host transposes/sums partials + bo.

Scheduling: input rows stream as separate pair/quarter tiles ordered
q.h0 / k.h0 / v.q0 / q.h1 / v.q1 / k.h1 (v h1 late); chunks run [0, 1,
3, 2] with the per-(chunk, head) filler schedule (remaining projections
fill chunks 0-1, O projections fill 3 and 2) to keep the PE dense for
the HAM clock gate; warm-up matmuls cover the initial DMA window; the
final O projection runs through the freed scores PSUM pool as m-block
pairs with ACT doing the output casts.
"""

import numpy as np
import ml_dtypes

import concourse.bacc as bacc
import concourse.mybir as mybir
import concourse.tile as tile
from concourse.bass_utils import run_bass_kernel_spmd

B, S, D, H = 2, 2048, 1024, 16
DK = D // H           # 64, head dim
DL = 256              # local (per-core) projected features = 4 heads
NHL = 4               # heads per core
NQ = 4                # q-chunks of 512
F32 = mybir.dt.float32
F16 = mybir.dt.float16
BF16 = mybir.dt.bfloat16
FP8 = mybir.dt.float8e4
E5 = mybir.dt.float8e5
U8 = mybir.dt.uint8
NPBF16 = ml_dtypes.bfloat16
NPFP8 = ml_dtypes.float8_e4m3
DRM = mybir.MatmulPerfMode.DoubleRow
# V/bv head-block permutation to slot order [h0, h2, h1, h3] (even-parity
# heads first): lets the PV matmul place each head's output directly in the
# partition half its ON slice needs.
VPERM = np.concatenate(
    [np.arange(64), np.arange(128, 192), np.arange(64, 128), np.arange(192, 256)]
)


def _emit(tc, io):
    nc = tc.nc
    qt, kt, vt = io["qt"], io["kt"], io["vt"]          # (1024, 2048) fp8
    qt16, kt16, vt16 = io["qt16"], io["kt16"], io["vt16"]  # (1024, 512) bf16
    wqt, wkt, wvt = io["wqt"], io["wkt"], io["wvt"]    # (128, 2048) fp8
    wqt16, wkt16, wvt16 = io["wqt16"], io["wkt16"], io["wvt16"]
    wot = io["wot"]                                    # (128, 2048) bf16
    bqc, bkc = io["bqc"], io["bkc"]                    # (128, 2) f32
    bvr = io["bvr"]                                    # (1, 256) bf16
    outp = io["outp"]                                  # (1024, 2048) f32
    EXP = mybir.ActivationFunctionType.Exp

    with (
        tc.tile_pool(name="const", bufs=1) as cw,
        tc.tile_pool(name="io", bufs=32) as iop,
        tc.tile_pool(name="big", bufs=1) as big,
        tc.tile_pool(name="work", bufs=3) as wk,
        tc.tile_pool(name="psA", bufs=2, space="PSUM") as psA,
        tc.tile_pool(name="psB", bufs=2, space="PSUM") as psB,
        tc.tile_pool(name="psC", bufs=2, space="PSUM") as psC,
    ):
        ones_sb = cw.tile([128, 128], BF16)
        nc.vector.memset(ones_sb[:], 1.0)
        # one-hot Z-broadcast lhsT: row 32h covers head h's 64-col block
        # (K=128 bf16; partition offsets stay 32-aligned for the BIR verifier)
        e4h = cw.tile([128, 256], BF16)
        nc.gpsimd.memset(e4h[:], 0.0)
        for hh in range(4):
            nc.gpsimd.memset(e4h[32 * hh : 32 * hh + 1, hh * 64 : (hh + 1) * 64], 1.0)
        # Z-row gather tiles (head h's Z on partition 32h) for the per-chunk
        # batched normalize; two statics alternate between in-flight chunks
        # (order 0,1,3,2 -> A,B,A,B) so unwritten rows stay deterministic
        zrowA = cw.tile([128, 512], F32)
        nc.gpsimd.memset(zrowA[:], 0.0)
        zrowB = cw.tile([128, 512], F32)
        nc.gpsimd.memset(zrowB[:], 0.0)
        bq_sb = cw.tile([128, 2], F32)
        nc.sync.dma_start(bq_sb[:], bqc[:, :])
        bk_sb = cw.tile([128, 2], F32)
        nc.sync.dma_start(bk_sb[:], bkc[:, :])
        bv_sb = cw.tile([1, 512], BF16)
        nc.sync.dma_start(bv_sb[:], bvr[:, :])

        wq_sb = cw.tile([128, 8, 256], FP8)

        # free PE warm-up: dependency-less matmuls run while the first
        # weight/row DMAs are in flight, so the HAM clock gate is already at
        # 8/8 when the real work begins
        warm = cw.tile([128, 512], BF16, name="warm")
        nc.vector.memset(warm[:], 0.0)
        for _ in range(4):
            wps = psC.tile([128, 512], F32, tag="pv", name="wps")
            nc.tensor.matmul(wps[:], ones_sb[:, :], warm[:], start=True, stop=True)

        QT = big.tile([128, 2, S], BF16)   # [feat%128, feat//128, seq]
        # K^T kept as two half-zeroed copies so the scores matmul contracts
        # over the full 128 partitions (zeros kill the other head's Q rows);
        # K=64 matmuls read as "half-idle" to the PE activity monitor and the
        # clock gate kept re-throttling the whole attention phase.
        KTe = big.tile([128, 2, S], BF16)
        KTo = big.tile([128, 2, S], BF16)
        # [key%128, slot, key//128, 128 cols] fp8: slots 0:2 hold the
        # even-parity heads (h=0,2) with feats in cols 0:64 + ones col at 64
        # (Z lands on pvp row 64); slots 2:4 hold odd-parity heads (h=1,3)
        # with feats in cols 64:128 + ones col at 0 (Z on pvp row 0) so the
        # PV output lands directly in the ON partition half it belongs to.
        # Padded to 128 cols so PV matmuls drive the full array.
        VA = big.tile([128, NHL, 16, 128], FP8)
        ON = big.tile([128, 2, S], BF16)   # normalized attn out, feature-major
        nc.gpsimd.memset(VA[:, 0:2, :, 64:128], 0.0)
        nc.gpsimd.memset(VA[:, 0:2, :, 64:65], 1.0)
        nc.gpsimd.memset(VA[:, 2:4, :, 0:64], 0.0)
        nc.gpsimd.memset(VA[:, 2:4, :, 0:1], 1.0)
        # bf16 copy of the first 4 key chunks for chunk 0's bf16 PV path
        VAB = big.tile([128, NHL, 4, 128], BF16)
        nc.gpsimd.memset(VAB[:, 0:2, :, 64:128], 0.0)
        nc.gpsimd.memset(VAB[:, 0:2, :, 64:65], 1.0)
        nc.gpsimd.memset(VAB[:, 2:4, :, 0:64], 0.0)
        nc.gpsimd.memset(VAB[:, 2:4, :, 0:1], 1.0)
        # K^T zero-halves AFTER the VA/VAB sets in the gpsimd queue: the VA
        # ones columns gate the first PV, the KT zeros only the first scores
        nc.gpsimd.memset(KTe[64:128, :, :], 0.0)
        nc.gpsimd.memset(KTo[0:64, :, :], 0.0)

        # ---- input row DMAs. Seq [0:512] of q/k/v streams in BF16 (with
        # bf16 weight copies): chunk 0's few-key softmax rows can't average
        # away fp8 noise, and clean K/V for keys 0:512 also cleans every
        # later chunk's scores against those keys. Seq [512:2048] streams as
        # fp8 k-tile PAIR tiles [128, 2, cols] (DoubleRow layout). Ordered by
        # first consumer; the v h1 halves are emitted late ----
        wq16_sb = cw.tile([128, 8, 256], BF16)
        wk16_sb = cw.tile([128, 8, 256], BF16)
        wv16_sb = cw.tile([128, 8, 256], BF16)
        wk_sb = cw.tile([128, 8, 256], FP8)
        wv_sb = cw.tile([128, 8, 256], FP8)
        wo_sb = cw.tile([128, 2, 1024], BF16)
        HS = 1024
        q16rows = [None] * 8   # [ktile] -> [128, 512] bf16 (seq 0:512)
        k16rows = [None] * 8
        v16rows = [None] * 8
        qrows = [[None, None] for _ in range(4)]   # [pair][0: seq 512:1024, 1: 1024:2048]
        krows = [[None, None] for _ in range(4)]
        vrows = [[None, None] for _ in range(4)]   # [pair][q1 512:1024, h1 1024:2048]

        def row16_dma(rows, src16, k, nm):
            r = iop.tile([128, 512], BF16, tag="x16", name=f"{nm}{k}", bufs=24)
            nc.sync.dma_start(r[:], src16[k * 128 : (k + 1) * 128, :])
            rows[k] = r

        def row_dma(rows, src, p, hf, nm):
            # hf 0: seq [512:1024] (w 512); hf 1: seq [1024:2048] (w 1024)
            w = 512 if hf == 0 else HS
            tg = "xq" if hf == 0 else "xrh"
            r = iop.tile([128, 2, w], FP8, tag=tg, name=f"{nm}{p}h{hf}", bufs=12)
            base = 512 if hf == 0 else 1024
            nc.sync.dma_start(
                r[:],
                src[p * 256 : (p + 1) * 256, base : base + w].rearrange(
                    "(i p) n -> p i n", i=2
                ),
            )
            rows[p][hf] = r

        nc.sync.dma_start(wq16_sb[:], wqt16[:, :].rearrange("p (k m) -> p k m", m=256))
        for k in range(8):
            row16_dma(q16rows, qt16, k, "q16r")
        nc.sync.dma_start(wk16_sb[:], wkt16[:, :].rearrange("p (k m) -> p k m", m=256))
        for k in range(8):
            row16_dma(k16rows, kt16, k, "k16r")
        nc.sync.dma_start(wv16_sb[:], wvt16[:, :].rearrange("p (k m) -> p k m", m=256))
        for k in range(8):
            row16_dma(v16rows, vt16, k, "v16r")
        nc.sync.dma_start(wq_sb[:], wqt[:, :].rearrange("p (k m) -> p k m", m=256))
        for p in range(4):
            row_dma(qrows, qt, p, 0, "qr")
        nc.sync.dma_start(wk_sb[:], wkt[:, :].rearrange("p (k m) -> p k m", m=256))
        for p in range(4):
            row_dma(krows, kt, p, 0, "kr")
        nc.sync.dma_start(wv_sb[:], wvt[:, :].rearrange("p (k m) -> p k m", m=256))
        for p in range(4):
            row_dma(vrows, vt, p, 0, "vr")
        for p in range(4):
            row_dma(qrows, qt, p, 1, "qr")
        for p in range(4):
            row_dma(krows, kt, p, 1, "kr")

        def xcol(rows, p, c0, w=512):  # [128, 2, w] fp8 slice (c0 >= 512)
            if c0 < 1024:
                return rows[p][0][:, :, c0 - 512 : c0 - 512 + w]
            return rows[p][1][:, :, c0 - 1024 : c0 - 1024 + w]

        def vcol(p, c0, w=128):  # fp8 v slice (c0 >= 512)
            if c0 < 1024:
                return vrows[p][0][:, :, c0 - 512 : c0 - 512 + w]
            return vrows[p][1][:, :, c0 - 1024 : c0 - 1024 + w]

        # ---- Q/K projections, feature-major: chunk 0 in bf16, chunks 1-3 as
        # fp8 DoubleRow (contract 2 k-tiles per matmul) ----
        def emit_qproj(n):
            pm = [
                psA.tile([128, 512], F32, tag="proj", name=f"pm{m}")
                for m in range(2)
            ]
            if n == 0:
                for k in range(8):
                    for m in range(2):
                        nc.tensor.matmul(
                            pm[m][:],
                            wq16_sb[:, k, m * 128 : (m + 1) * 128],
                            q16rows[k][:],
                            start=(k == 0),
                            stop=(k == 7),
                        )
            else:
                for p in range(4):
                    for m in range(2):
                        nc.tensor.matmul(
                            pm[m][:],
                            wq_sb[:, 2 * p : 2 * p + 2, m * 128 : (m + 1) * 128],
                            xcol(qrows, p, n * 512),
                            start=(p == 0),
                            stop=(p == 3),
                            perf_mode=DRM,
                        )
            for m in range(2):
                nc.vector.tensor_scalar_add(
                    QT[:, m, n * 512 : (n + 1) * 512], pm[m][:], bq_sb[:, m : m + 1]
                )

        # K projection split per q-chunk: attention chunk j only needs K
        # columns up to (j+1)*512, so later chunks are emitted between the
        # attention chunks below (PE-dense filler for the exp-paced phase).
        # Bias adds stay on DVE: GpSimd cannot read PSUM.
        def emit_kproj(n):
            pm = [
                psA.tile([128, 512], F32, tag="proj", name=f"km{m}")
                for m in range(2)
            ]
            if n == 0:
                for k in range(8):
                    for m in range(2):
                        nc.tensor.matmul(
                            pm[m][:],
                            wk16_sb[:, k, m * 128 : (m + 1) * 128],
                            k16rows[k][:],
                            start=(k == 0),
                            stop=(k == 7),
                        )
            else:
                for p in range(4):
                    for m in range(2):
                        nc.tensor.matmul(
                            pm[m][:],
                            wk_sb[:, 2 * p : 2 * p + 2, m * 128 : (m + 1) * 128],
                            xcol(krows, p, n * 512),
                            start=(p == 0),
                            stop=(p == 3),
                            perf_mode=DRM,
                        )
            for m in range(2):
                sl = slice(n * 512, (n + 1) * 512)
                nc.vector.tensor_scalar_add(
                    KTe[0:64, m, sl], pm[m][0:64, :], bk_sb[0:64, m : m + 1]
                )
                nc.vector.tensor_scalar_add(
                    KTo[64:128, m, sl], pm[m][64:128, :], bk_sb[64:128, m : m + 1]
                )

        emit_qproj(0)
        emit_kproj(0)
        for _ in range(8):  # filler: the v16 rows may still be in flight
            wps = psC.tile([128, 512], F32, tag="pv", name="wps")
            nc.tensor.matmul(wps[:], ones_sb[:, :], warm[:], start=True, stop=True)
        nc.sync.dma_start(wo_sb[:], wot[:, :].rearrange("p (c m) -> p c m", m=1024))

        # ---- V projection, natural layout, DoubleRow (x rows stationary),
        # bias via K=1 bf16 ones matmul; emitted in sp-pairs interleaved with
        # the attention chunks below as PE-dense filler ----
        def emit_vproj(sps):
            for sp in sps:
                pvps = psA.tile([128, 512], F32, tag="proj", name="pvps")
                if sp < 2:  # seq 0:512 in bf16 (x rows stationary)
                    for half in range(2):
                        s = sp * 256 + half * 128
                        for k in range(8):
                            nc.tensor.matmul(
                                pvps[:, half * 256 : (half + 1) * 256],
                                v16rows[k][:, s : s + 128],
                                wv16_sb[:, k, :],
                                start=(k == 0 and half == 0),
                                stop=False,
                                skip_group_check=(half == 1),
                            )
                else:
                    for half in range(2):
                        s = sp * 256 + half * 128
                        for p in range(4):
                            nc.tensor.matmul(
                                pvps[:, half * 256 : (half + 1) * 256],
                                vcol(p, s),
                                wv_sb[:, 2 * p : 2 * p + 2, :],
                                start=(p == 0 and half == 0),
                                stop=False,
                                perf_mode=DRM,
                                skip_group_check=(half == 1),
                            )
                nc.tensor.matmul(
                    pvps[:, 0:512],
                    ones_sb[0:1, 0:128],
                    bv_sb[:],
                    start=False,
                    stop=True,
                    skip_group_check=True,
                )
                # wvt/bvr are host-permuted to slot order [h0, h2, h1, h3]:
                # slots 0:2 (even-parity heads) land in VA cols 0:64, slots
                # 2:4 (odd-parity) in cols 64:128 — matching the pvp row
                # ranges their ON partition halves need.
                for half in range(2):
                    s = sp * 2 + half
                    nc.vector.tensor_copy(
                        VA[:, 0:2, s, 0:64],
                        pvps[:, half * 256 : half * 256 + 128].rearrange(
                            "p (h d) -> p h d", d=64
                        ),
                    )
                    nc.vector.tensor_copy(
                        VA[:, 2:4, s, 64:128],
                        pvps[:, half * 256 + 128 : (half + 1) * 256].rearrange(
                            "p (h d) -> p h d", d=64
                        ),
                    )
                    if s < 4:  # chunk 0's bf16 PV needs these in bf16 too
                        nc.vector.tensor_copy(
                            VAB[:, 0:2, s, 0:64],
                            pvps[:, half * 256 : half * 256 + 128].rearrange(
                                "p (h d) -> p h d", d=64
                            ),
                        )
                        nc.vector.tensor_copy(
                            VAB[:, 2:4, s, 64:128],
                            pvps[:, half * 256 + 128 : (half + 1) * 256].rearrange(
                                "p (h d) -> p h d", d=64
                            ),
                        )

        # ---- attention + output projection, q-chunk-major for overlap ----
        # (vproj(0,1) is emitted inside head (0,0), between its scores/exp
        # and its PV matmuls, so the scores don't queue behind the v16 DMA)

        def emit_oproj(j, ms=range(8)):
            for m in ms:
                po = psA.tile([128, 512], F32, tag="proj", name="po")
                for c in range(2):
                    nc.tensor.matmul(
                        po[:],
                        wo_sb[:, c, m * 128 : (m + 1) * 128],
                        ON[:, c, j * 512 : (j + 1) * 512],
                        start=(c == 0),
                        stop=(c == 1),
                    )
                ot = wk.tile([128, 512], F16, tag="ot", name="ot")
                nc.vector.tensor_copy(ot[:], po[:])
                nc.sync.dma_start(
                    outp[m * 128 : (m + 1) * 128, j * 512 : (j + 1) * 512], ot[:]
                )

        # normalize chunk j (batched over its 4 heads): gpsimd gathers the 4
        # Z rows into zrow, ONE 32x32 stream-transpose spreads all of them
        # across 32 lanes, reciprocal runs on a strided view (4 cols per
        # 32-block), bf16 cast + transpose-back, then a K=32 one-hot bf16
        # matmul per head broadcasts 1/Z across the head's 64 partitions and
        # DVE scales u into ON. Odd-parity heads produced their PV output
        # directly on partitions 64:128 (VA slot layout), so both parities
        # write ON in place.
        def emit_norm_chunk(j, us):
            sl = slice(j * 512, (j + 1) * 512)
            zrow = zrowA if j in (0, 3) else zrowB
            zeng = nc.vector if j == 2 else nc.gpsimd
            for h in range(4):
                zb = 64 if h % 2 == 0 else 0
                zeng.tensor_copy(zrow[32 * h : 32 * h + 1, :], us[h][zb : zb + 1, :])
            zt = wk.tile([128, 512], F32, tag="zt", name="zt", bufs=2)
            nc.vector.transpose(zt[:, :], zrow[:, :])
            nc.vector.reciprocal(zt[:, 0:512:32], zt[:, 0:512:32])
            ztb = wk.tile([128, 512], BF16, tag="ztb", name="ztb", bufs=2)
            nc.vector.tensor_copy(ztb[:, :], zt[:, :])
            zbb = wk.tile([128, 512], BF16, tag="zbb", name="zbb", bufs=2)
            nc.vector.transpose(zbb[:, :], ztb[:, :])
            for ch in range(2):
                he, ho = 2 * ch, 2 * ch + 1
                rbt = psA.tile([128, 512], F32, tag="proj", name="rbt")
                nc.tensor.matmul(
                    rbt[0:64, :], e4h[:, he * 64 : (he + 1) * 64], zbb[:, :],
                    start=True, stop=True,
                )
                nc.tensor.matmul(
                    rbt[64:128, :], e4h[:, ho * 64 : (ho + 1) * 64], zbb[:, :],
                    start=True, stop=True, skip_group_check=True,
                )
                nc.vector.tensor_mul(ON[0:64, ch, sl], us[he][0:64, :], rbt[0:64, :])
                nc.vector.tensor_mul(
                    ON[64:128, ch, sl], us[ho][64:128, :], rbt[64:128, :]
                )

        # half-chunk normalize for the tail chunk: heads (0,1) [pb=0] or
        # (2,3) [pb=64] only — halves the final DVE chain so the tail
        # O-projection starts sooner. Partition math mirrors the full chunk
        # version; all offsets stay 32-aligned.
        def emit_norm_half(j, heads, us):
            sl = slice(j * 512, (j + 1) * 512)
            zrow = zrowA if j in (0, 3) else zrowB
            pb = 0 if heads[0] == 0 else 64
            for h in heads:
                zb = 64 if h % 2 == 0 else 0
                nc.vector.tensor_copy(
                    zrow[32 * h : 32 * h + 1, :], us[h][zb : zb + 1, :]
                )
            zt = wk.tile([128, 512], F32, tag="zt", name="zt", bufs=2)
            nc.vector.transpose(zt[pb : pb + 64, :], zrow[pb : pb + 64, :])
            nc.vector.reciprocal(
                zt[pb : pb + 64, 0:512:32], zt[pb : pb + 64, 0:512:32]
            )
            ztb = wk.tile([128, 512], BF16, tag="ztb", name="ztb", bufs=2)
            nc.vector.tensor_copy(ztb[pb : pb + 64, :], zt[pb : pb + 64, :])
            zbb = wk.tile([128, 512], BF16, tag="zbb", name="zbb", bufs=2)
            nc.vector.transpose(zbb[pb : pb + 64, :], ztb[pb : pb + 64, :])
            he, ho = heads
            ch = he // 2
            rbt = psA.tile([128, 512], F32, tag="proj", name="rbt")
            nc.tensor.matmul(
                rbt[0:64, :], e4h[pb : pb + 64, he * 64 : (he + 1) * 64],
                zbb[pb : pb + 64, :], start=True, stop=True,
            )
            nc.tensor.matmul(
                rbt[64:128, :], e4h[pb : pb + 64, ho * 64 : (ho + 1) * 64],
                zbb[pb : pb + 64, :], start=True, stop=True,
                skip_group_check=True,
            )
            nc.vector.tensor_mul(ON[0:64, ch, sl], us[he][0:64, :], rbt[0:64, :])
            nc.vector.tensor_mul(
                ON[64:128, ch, sl], us[ho][64:128, :], rbt[64:128, :]
            )

        # PE filler schedule: chunks run in order [0, 1, 3, 2] so the big
        # ACT-paced late chunks have O-projections available as filler (chunk
        # 2 last gets oproj(3)); chunk 0's fillers finish the Q projection
        # (attention j only reads QT chunk j) and the K/V pipeline. Each
        # (chunk, head) entry is emitted right after that head's PV drain.
        def fillers(j, h):
            if (j, h) == (0, 0):
                emit_qproj(1)
            elif (j, h) == (0, 1):
                emit_kproj(1)
            elif (j, h) == (0, 2):
                emit_qproj(2)
            elif (j, h) == (0, 3):
                for p in range(4):  # late v halves recycle released q slots
                    r = iop.tile([128, 2, HS], FP8, tag="xrh", name=f"vr{p}h1", bufs=12)
                    nc.sync.dma_start(
                        r[:],
                        vt[p * 256 : (p + 1) * 256, 1024:2048].rearrange(
                            "(i p) n -> p i n", i=2
                        ),
                    )
                    vrows[p][1] = r
                emit_vproj((2, 3))
            elif (j, h) == (1, 0):
                emit_kproj(2)
            elif (j, h) == (1, 1):
                emit_qproj(3)
            elif (j, h) == (1, 2):
                emit_kproj(3)
            elif (j, h) == (1, 3):
                emit_vproj((4, 5, 6, 7))
            elif (j, h) == (3, 0):
                emit_oproj(0, range(4))
            elif (j, h) == (3, 1):
                emit_oproj(0, range(4, 8))
            elif (j, h) == (3, 2):
                emit_oproj(1, range(4))
            elif (j, h) == (3, 3):
                emit_oproj(1, range(4, 8))
                for _ in range(6):  # cover norm(3)'s DVE-chain wait
                    wps = psC.tile([128, 512], F32, tag="pv", name="wps")
                    nc.tensor.matmul(
                        wps[:], ones_sb[:, :], warm[:], start=True, stop=True
                    )
            elif (j, h) == (2, 0):
                emit_oproj(3, range(4))
            elif (j, h) == (2, 1):
                emit_oproj(3, range(4, 8))
            elif (j, h) in ((2, 2), (2, 3)):
                # dependency-free PE keep-alive: bridges the chunk-2 norm
                # window so the HAM clock gate doesn't throttle the tail
                # O-projection to half speed
                for _ in range(6 if h == 2 else 26):
                    wps = psC.tile([128, 512], F32, tag="pv", name="wps")
                    nc.tensor.matmul(
                        wps[:], ones_sb[:, :], warm[:], start=True, stop=True
                    )

        # chunks 0 and 1 run head-sequentially (their fillers are the
        # remaining projections); chunks 3 and 2 INTERLEAVE heads so their
        # pooled PE deficit draws on the pooled O-projection filler
        SEQ = [(0, h) for h in range(NHL)] + [(1, h) for h in range(NHL)]
        SEQ += [(3, h) for h in range(NHL)] + [(2, h) for h in range(NHL)]
        # norm of chunk c is emitted after the u-copy at position (j, h) —
        # late enough not to block the next chunk's VA/bias DVE work, early
        # enough that the O-projection fillers find ON ready
        NORM_AT = {(1, 0): 0, (3, 0): 1}
        # (3,3) norm runs before chunk 2's attention begins, so the oproj(3)
        # fillers at (2,0)/(2,1) find ON(3) ready
        uchunk = {}
        for j, h in SEQ:
            nkc = 4 * (j + 1)
            ch = h // 2
            slot = (h % 2) * 2 + ch
            pvp = psC.tile([128, 512], F32, tag="pv", name="pvp")
            first = (j, h) == (0, 0)
            ets = {}
            for kp in range(nkc // 2):
                scp = psB.tile([128, 1024], F32, tag="sc", name="scp")
                ktz = KTe if h % 2 == 0 else KTo
                # q0 per half (diagonal chunk t = kc-4j only reaches q >=
                # 128t); the PAIR is processed over [q0p:512] where q0p is
                # the first half's q0 — the widened affine_select (negative
                # base) zeroes the second half's fully-masked leading cols.
                q0 = [128 * max(0, 2 * kp + half - 4 * j) for half in range(2)]
                q0p = q0[0]
                for half in range(2):
                    kc = 2 * kp + half
                    nc.tensor.matmul(
                        scp[:, half * 512 + q0[half] : (half + 1) * 512],
                        ktz[:, ch, kc * 128 : (kc + 1) * 128],
                        QT[:, ch, j * 512 + q0[half] : (j + 1) * 512],
                        start=True,
                        stop=True,
                    )
                # chunk 0's q rows see few keys, so softmax averaging can't
                # wash out fp8 quantization noise — use bf16 et + per-kc bf16
                # PV (VAB) for j=0; fp8e5 et + DoubleRow PV elsewhere
                edt = BF16 if j == 0 else E5
                et = wk.tile([128, 1024], edt, tag="exp" + ("16" if j == 0 else ""),
                             name="et", bufs=4 if j == 0 else 8)
                if q0p == 0 and q0[1] == 0:
                    if j == 3 and kp == 5:
                        # Schraudolph exp on DVE (affine -> uint8, bitcast
                        # e5m2; negatives saturate to +0) — sheds one ACT
                        # instruction per chunk-3 head onto the idler DVE
                        nc.vector.tensor_scalar(
                            et[:].bitcast(U8), scp[:], 0.7213475, 59.8,
                            mybir.AluOpType.mult, mybir.AluOpType.add,
                        )
                    else:
                        nc.scalar.activation(
                            et[:], scp[:], EXP, scale=float(DK) ** -0.5
                        )
                else:
                    # one ACT instruction covers both halves over [q0p:512]
                    # (strided 2D-free AP); the [q0p:q0[1]] region of the
                    # second half is exp(PSUM garbage) — finite, and zeroed
                    # by the widened select below
                    etv = et[:, :].rearrange("p (c n) -> p c n", c=2)
                    scv = scp[:, :].rearrange("p (c n) -> p c n", c=2)
                    nc.scalar.activation(
                        etv[:, :, q0p:512],
                        scv[:, :, q0p:512],
                        EXP,
                        scale=float(DK) ** -0.5,
                    )
                for half in range(2):
                    kc = 2 * kp + half
                    if kc >= 4 * j:
                        t = kc - 4 * j
                        nc.gpsimd.affine_select(
                            out=et[:, half * 512 + q0p : (half + 1) * 512],
                            in_=et[:, half * 512 + q0p : (half + 1) * 512],
                            compare_op=mybir.AluOpType.is_ge,
                            fill=0.0,
                            base=q0p - 128 * t,
                            pattern=[[1, 512 - q0p]],
                            channel_multiplier=-1,
                        )
                if first:
                    ets[kp] = (et, list(q0))
                    continue
                if j == 0:
                    for half in range(2):
                        kc = 2 * kp + half
                        nc.tensor.matmul(
                            pvp[:, q0[half] : 512],
                            VAB[:, slot, kc, :],
                            et[:, half * 512 + q0[half] : (half + 1) * 512],
                            start=(kc == 0),
                            stop=(kc == nkc - 1),
                            skip_group_check=(q0[half] > 0),
                        )
                else:
                    # PV DoubleRow: one matmul contracts both key chunks of
                    # the pair (VA slot already holds [keys, kc, 128cols] fp8)
                    etv = et[:, :].rearrange("p (c n) -> p c n", c=2)
                    nc.tensor.matmul(
                        pvp[:, q0p:512],
                        VA[:, slot, 2 * kp : 2 * kp + 2, :],
                        etv[:, :, q0p:512],
                        start=(kp == 0),
                        stop=(kp == nkc // 2 - 1),
                        perf_mode=DRM,
                        skip_group_check=(q0p > 0),
                    )
            if first:
                emit_vproj((0, 1))
                for kp in range(nkc // 2):
                    et, q0 = ets[kp]
                    for half in range(2):
                        kc = 2 * kp + half
                        nc.tensor.matmul(
                            pvp[:, q0[half] : 512],
                            VAB[:, slot, kc, :],
                            et[:, half * 512 + q0[half] : (half + 1) * 512],
                            start=(kc == 0),
                            stop=(kc == nkc - 1),
                            skip_group_check=(q0[half] > 0),
                        )
            # copy the PV accumulator to SBUF immediately — freeing the
            # pv PSUM slot fast keeps the PE from stalling. The normalize is
            # deferred by ONE head so the next head's PSUM-freeing copy is
            # never queued behind it on DVE.
            u = wk.tile([128, 512], F32, tag="u", name="u", bufs=9)
            nc.vector.tensor_copy(u[:], pvp[:])
            uchunk.setdefault(j, {})[h] = u
            # fillers BEFORE the norm: the norm's broadcast matmuls wait on
            # the DVE transpose/recip chain, and the in-order PE queue would
            # stall the filler matmuls behind them
            fillers(j, h)
            if (j, h) in NORM_AT:
                emit_norm_chunk(NORM_AT[(j, h)], uchunk.pop(NORM_AT[(j, h)]))
            elif (j, h) == (3, 2):
                emit_norm_half(3, (0, 1), uchunk[3])
            elif (j, h) == (3, 3):
                emit_norm_half(3, (2, 3), uchunk.pop(3))
            elif (j, h) == (2, 1):
                emit_norm_half(2, (0, 1), uchunk[2])
            elif (j, h) == (2, 3):
                emit_norm_half(2, (2, 3), uchunk.pop(2))
        # keep-alive while chunk 2's ON multiplies drain on DVE (the first
        # tail matmul below waits on them; an idle PE would re-throttle)
        for _ in range(6):
            wps = psC.tile([128, 512], F32, tag="pv", name="wps")
            nc.tensor.matmul(wps[:], ones_sb[:, :], warm[:], start=True, stop=True)
        # last chunk's (j=2) O-projection: psB (the scores pool) is free now,
        # so build m-block PAIRS in [128,1024] tiles — half the copies/DMAs
        # and 2KB/partition output descriptors — to compress the serial tail.
        j = 2
        for mp in range(4):
            po = psB.tile([128, 1024], F32, tag="sc", name="po2")
            for i in range(2):
                m = 2 * mp + i
                for c in range(2):
                    nc.tensor.matmul(
                        po[:, i * 512 : (i + 1) * 512],
                        wo_sb[:, c, m * 128 : (m + 1) * 128],
                        ON[:, c, j * 512 : (j + 1) * 512],
                        start=(c == 0),
                        stop=(c == 1),
                    )
            ot = wk.tile([128, 1024], F16, tag="ot2", name="ot2", bufs=2)
            nc.scalar.copy(ot[:], po[:])  # ACT is idle at the tail; DVE isn't
            nc.sync.dma_start(
                outp[
                    2 * mp * 128 : (2 * mp + 2) * 128, j * 512 : (j + 1) * 512
                ].rearrange("(b p) c -> p b c", p=128),
                ot[:].rearrange("p (b c) -> p b c", c=512),
            )


def build_nc():
    nc = bacc.Bacc("TRN2", target_bir_lowering=False, debug=False, num_devices=8)
    io = {}
    for name, shape, dt in (
        ("qt", (D, S), FP8),
        ("kt", (D, S), FP8),
        ("vt", (D, S), FP8),
        ("qt16", (D, 512), BF16),
        ("kt16", (D, 512), BF16),
        ("vt16", (D, 512), BF16),
        ("wqt16", (128, 2048), BF16),
        ("wkt16", (128, 2048), BF16),
        ("wvt16", (128, 2048), BF16),
        ("wqt", (128, 2048), FP8),
        ("wkt", (128, 2048), FP8),
        ("wvt", (128, 2048), FP8),
        ("wot", (128, 2048), BF16),
        ("bqc", (128, 2), F32),
        ("bkc", (128, 2), F32),
        ("bvr", (1, 512), BF16),
    ):
        io[name] = nc.dram_tensor(name, shape, dt, kind="ExternalInput")
    io["outp"] = nc.dram_tensor("outp", (D, S), F16, kind="ExternalOutput")
    with tile.TileContext(nc) as tc:
        _emit(tc, io)
    nc.compile()
    return nc


_NC = None


def _get_nc():
    global _NC
    if _NC is None:
        _NC = build_nc()
    return _NC


def make_in_maps(q, k, v, Wq, bq, Wk, bk, Wv, bv, Wo):
    def c8(x):  # contiguous fp8e4
        return np.ascontiguousarray(x).astype(NPFP8)

    def cb(x):  # contiguous bf16
        return np.ascontiguousarray(x).astype(NPBF16)

    cf = np.ascontiguousarray
    in_maps = []
    for core in range(8):
        b, g = divmod(core, 4)
        sl = slice(DL * g, DL * (g + 1))
        in_maps.append(
            {
                "qt": c8(q[b].T),
                "kt": c8(k[b].T),
                "vt": c8(v[b].T),
                "qt16": cb(q[b].T[:, 0:512]),
                "kt16": cb(k[b].T[:, 0:512]),
                "vt16": cb(v[b].T[:, 0:512]),
                "wqt16": cb(Wq[sl, :].T.reshape(8, 128, DL).transpose(1, 0, 2).reshape(128, 2048)),
                "wkt16": cb(Wk[sl, :].T.reshape(8, 128, DL).transpose(1, 0, 2).reshape(128, 2048)),
                "wvt16": cb(Wv[sl, :][VPERM, :].T.reshape(8, 128, DL).transpose(1, 0, 2).reshape(128, 2048)),
                "wqt": c8(Wq[sl, :].T.reshape(8, 128, DL).transpose(1, 0, 2).reshape(128, 2048)),
                "wkt": c8(Wk[sl, :].T.reshape(8, 128, DL).transpose(1, 0, 2).reshape(128, 2048)),
                "wvt": c8(Wv[sl, :][VPERM, :].T.reshape(8, 128, DL).transpose(1, 0, 2).reshape(128, 2048)),
                "wot": cb(Wo[:, sl].T.reshape(2, 128, D).transpose(1, 0, 2).reshape(128, 2048)),
                "bqc": cf(bq[sl].reshape(2, 128).T),
                "bkc": cf(bk[sl].reshape(2, 128).T),
                "bvr": cb(np.tile(bv[sl][VPERM], 2).reshape(1, 512)),
            }
        )
    return in_maps


def gather_output(results, bo):
    out = np.empty((B, S, D), np.float32)
    for b in range(B):
        acc = results[4 * b]["outp"].astype(np.float32)
        for g in range(1, 4):
            acc = acc + results[4 * b + g]["outp"]
        out[b] = acc.T + bo
    return out


def _np_fallback(q, k, v, mask, Wq, bq, Wk, bk, Wv, bv, Wo, bo):
    # generic-mask reference path; only used if the mask is not causal
    out = np.empty((B, S, D), np.float32)
    m = np.broadcast_to(mask, (B, 1, S, S))
    for b in range(B):
        Q = (q[b] @ Wq.T + bq).reshape(S, H, DK).transpose(1, 0, 2)
        K = (k[b] @ Wk.T + bk).reshape(S, H, DK).transpose(1, 0, 2)
        V = (v[b] @ Wv.T + bv).reshape(S, H, DK).transpose(1, 0, 2)
        o = np.empty((H, S, DK), np.float32)
        for hh in range(H):
            s = (Q[hh] @ K[hh].T) * (DK**-0.5)
            s = np.where(m[b, 0] == 0, -np.inf, s)
            s = s - s.max(axis=-1, keepdims=True)
            e = np.exp(s)
            o[hh] = (e / e.sum(axis=-1, keepdims=True)) @ V[hh]
        out[b] = o.transpose(1, 0, 2).reshape(S, D) @ Wo.T + bo
    return out


def kernel(q, k, v, mask, Wq, bq, Wk, bk, Wv, bv, Wo, bo):
    f32 = np.float32
    q, k, v = (np.asarray(x, f32) for x in (q, k, v))
    Wq, bq, Wk, bk = (np.asarray(x, f32) for x in (Wq, bq, Wk, bk))
    Wv, bv, Wo, bo = (np.asarray(x, f32) for x in (Wv, bv, Wo, bo))
    mask = np.asarray(mask)

    if not np.array_equal(
        np.broadcast_to(mask, (1, 1, S, S))[0, 0] != 0,
        np.tril(np.ones((S, S), bool)),
    ):
        return _np_fallback(q, k, v, mask, Wq, bq, Wk, bk, Wv, bv, Wo, bo)

    nc = _get_nc()
    in_maps = make_in_maps(q, k, v, Wq, bq, Wk, bk, Wv, bv, Wo)
    res = run_bass_kernel_spmd(nc, in_maps, list(range(8)))
    return gather_output(res.results, bo)
